# revision 10
# baseline (speedup 1.0000x reference)
"""Trainium2 Bass kernel for nn_Contraction_627065225897 (MACE-style symmetric
contraction with per-element (MoE-routed) weights).

Math (per atom n with element e = sorted_indices[n], channel f):
  out[n,f] = sum_p W3[e,p,f] * T3[n,f,p]  +  sum_q W2[e,q,f] * T2[n,f,q]
           + W1[e,0,f] * T1[n,f]
  T3[n,f,p]   = sum_{m1,m2,k} U3[p,m1,m2,k] x_m1 x_m2 x_k   (+ lower orders)

Device strategy (per core, 16 atoms, f=128 on output partitions):
  - host ships x pre-replicated in (m2,k)-partition layout (XK/XM, bf16) in
    two half-chunks each, so xxT[(m2,k), (n,f)] = XM * XK is a packed
    elementwise bf16 multiply on DVE (2x) -- no outer products, no PE
    transposes, no PSUM evac for xx
  - PE matmuls (bf16): t3[f, 4q+368pm] = xxT_n.T @ MOV, 2 chained per atom
  - W[f, n, 372] = [w2s | W3[e_n] (x) x] built with stride-0 broadcast APs
    (DVE/GPSIMD split), bf16
  - Act evacuates t3 PSUM->SBUF per 4-atom group, converting to bf16
  - fused reduce per group: Z = ts * W (packed bf16, DVE 2x) then per-atom
    tensor_scalar accumulate (DVE 4x) -> ACCF
  - linear term: M = x * (U1 (x) W1) on GPSIMD + grouped tensor_reduce (DVE)
  - output written in two halves to overlap the final DMA with compute
Sharding: data-parallel over atoms, 16 atoms/core on 8 cores; per-element
weights gathered host-side by sorted_indices (routing) and replicated.
"""

import os
import sys
from contextlib import ExitStack

import numpy as np

if "/opt/trn_rl_repo" not in sys.path:
    sys.path.insert(0, "/opt/trn_rl_repo")

B, F, L = 128, 128, 16
E = 10
P3, P2, P1 = 23, 4, 1
NCORES = 8
BS = B // NCORES  # atoms per core = 16
NPM = P3 * L  # 368 cubic columns (p-major, m1-minor)
NCOL = P2 + NPM  # 372 moving columns: [quad(4) | cubic(368)]
NG = 4  # atoms per t3 PSUM group (4 banks/group)
NF = BS * F  # 2048 (n,f) columns in replicated layout
HF = NF // 2  # half of the (n,f) columns (atoms 0-7 / 8-15)
# wsmall packing: [ xsb(256) | cw3(368) | w2s(64) | u1wm(256) ]
WS_XSB, WS_CW3, WS_W2S, WS_U1 = 0, 256, 624, 688
WS_TOT = 944

_CACHE = {}


def _build_program(cfg_key):
    import concourse.bass as bass
    import concourse.mybir as mybir
    import concourse.tile as tile
    from concourse import bacc

    dt = mybir.dt.float32
    db = mybir.dt.bfloat16
    nc = bacc.Bacc("TRN2", target_bir_lowering=False, debug=False)

    # W-build batches (of 4 atoms) on GPSIMD instead of DVE
    w_pool = set(
        int(s) for s in os.environ.get("KERNEL_W_POOL", "2,3").split(",") if s
    )

    ws_d = nc.dram_tensor("wsmall", [128, WS_TOT], db, kind="ExternalInput")
    mov_d = nc.dram_tensor("mov", [128, 2 * NCOL], db, kind="ExternalInput")
    xk_d = nc.dram_tensor("xk", [128, NF], db, kind="ExternalInput")
    xm0_d = nc.dram_tensor("xm0", [128, NF], db, kind="ExternalInput")
    xm1_d = nc.dram_tensor("xm1", [128, NF], db, kind="ExternalInput")
    out_d = nc.dram_tensor("outT", [128, BS], dt, kind="ExternalOutput")

    mult = mybir.AluOpType.mult
    add = mybir.AluOpType.add

    with tile.TileContext(nc) as tc, ExitStack() as ctx:
        const = ctx.enter_context(tc.tile_pool(name="const", bufs=1))
        work = ctx.enter_context(tc.tile_pool(name="work", bufs=3))
        ps_t3 = ctx.enter_context(
            tc.tile_pool(name="ps_t3", bufs=2, space=bass.MemorySpace.PSUM)
        )

        WS = const.tile([128, WS_TOT], db)
        XK = const.tile([128, NF], db)
        XM0 = const.tile([128, NF], db)
        XM1 = const.tile([128, NF], db)
        MOV = const.tile([128, 2 * NCOL], db)

        # All DMAs on the SP queue, ordered by need time; the replicated-x
        # tensors are split in half-chunks so the first matmuls start early.
        nc.sync.dma_start(XK[:, 0:HF], xk_d.ap()[:, 0:HF])
        nc.sync.dma_start(XM0[:, 0:HF], xm0_d.ap()[:, 0:HF])
        nc.sync.dma_start(XM1[:, 0:HF], xm1_d.ap()[:, 0:HF])
        nc.sync.dma_start(MOV[:], mov_d.ap())
        nc.sync.dma_start(WS[:], ws_d.ap())
        nc.sync.dma_start(XK[:, HF:NF], xk_d.ap()[:, HF:NF])
        nc.sync.dma_start(XM0[:, HF:NF], xm0_d.ap()[:, HF:NF])
        nc.sync.dma_start(XM1[:, HF:NF], xm1_d.ap()[:, HF:NF])

        # PE p-state warm-up: dummy matmuls on a memset tile keep the PE busy
        # from t~0.7us so the real matmuls run at full clock (3us ramp).
        WARM = const.tile([128, 128], db)
        nc.gpsimd.memset(WARM[:], 0.0)
        warm_ps = ps_t3.tile([128, NG * 512], dt, tag="t3g")
        wmov = WARM[:].unsqueeze(1).broadcast_to((128, 4, 128))
        for _ in range(7):
            nc.tensor.matmul(warm_ps[:, 0:512], WARM[:], wmov, start=True, stop=True)

        XSB = WS[:, WS_XSB : WS_XSB + BS * L]
        CW3 = WS[:, WS_CW3 : WS_CW3 + BS * P3]
        U1WM = WS[:, WS_U1 : WS_U1 + BS * L]
        MOV0 = MOV[:, 0:NCOL]
        MOV1 = MOV[:, NCOL : 2 * NCOL]

        WALL = const.tile([128, BS * NCOL], db)
        WALL_v = WALL[:].rearrange("p (n c) -> p n c", n=BS)
        # quad weight cols [0:4) copied from wsmall (tiny DVE copy)
        wq = work.tile([128, BS * P2], db, tag="wq")
        nc.vector.tensor_scalar(
            out=WALL_v[:, :, 0:P2],
            in0=WS[:, WS_W2S : WS_W2S + BS * P2].rearrange("p (n c) -> p n c", n=BS),
            scalar1=1.0,
            scalar2=0.0,
            op0=mult,
            op1=add,
        )

        XXT0 = const.tile([128, NF], db)  # xxT half 0: partitions (m2<8, k)
        XXT1 = const.tile([128, NF], db)  # xxT half 1: partitions (m2>=8, k)
        ACCF = const.tile([128, BS], dt)  # fused quad+cubic accums
        ACCL = const.tile([128, BS], dt)  # linear accums
        OUT = const.tile([128, BS], dt)

        # ---- W[f, n, 4+(p,m1)] = W3[e_n,p,f] * x[n,f,m1] (stride-0 APs) ----
        def build_w(b):
            wv = WALL_v[:, b * NG : (b + 1) * NG, P2:NCOL].rearrange(
                "p n (a b) -> p n a b", a=P3
            )
            c0 = (
                CW3[:, b * NG * P3 : (b + 1) * NG * P3]
                .rearrange("p (n a) -> p n a", n=NG)
                .unsqueeze(3)
                .broadcast_to((128, NG, P3, L))
            )
            x0 = (
                XSB[:, b * NG * L : (b + 1) * NG * L]
                .rearrange("p (n a) -> p n a", n=NG)
                .unsqueeze(2)
                .broadcast_to((128, NG, P3, L))
            )
            weng = nc.gpsimd if b in w_pool else nc.vector
            weng.tensor_tensor(wv, c0, x0, op=mult)

        # xxT for one half h of the atoms (8 atoms = 1024 cols)
        def build_xxt(h):
            sl = slice(h * HF, (h + 1) * HF)
            nc.vector.tensor_tensor(XXT0[:, sl], XM0[:, sl], XK[:, sl], op=mult)
            nc.vector.tensor_tensor(XXT1[:, sl], XM1[:, sl], XK[:, sl], op=mult)

        # Pool builds its W batches early; DVE interleaves xxT and W
        for b in sorted(w_pool):
            build_w(b)
        build_xxt(0)
        for b in range(BS // NG):
            if b not in w_pool:
                build_w(b)
        build_xxt(1)

        # ---- linear term: M = x * (U1 (x) W1), grouped reduce over l ----
        MLIN = const.tile([128, BS * L], dt)
        nc.gpsimd.tensor_tensor(MLIN[:], XSB, U1WM, op=mult)
        nc.vector.tensor_reduce(
            ACCL[:],
            MLIN[:].rearrange("p (n l) -> p n l", n=BS),
            axis=mybir.AxisListType.X,
            op=add,
        )

        # ---- PE matmuls -> Act group evac (bf16) -> DVE Z + 4x accumulate ----
        for g in range(BS // NG):
            t3g = ps_t3.tile([128, NG * 512], dt, tag="t3g")
            for j in range(NG):
                n = g * NG + j
                nc.tensor.matmul(
                    t3g[:, j * 512 : j * 512 + NCOL],
                    XXT0[:, n * F : (n + 1) * F],
                    MOV0,
                    start=True,
                    stop=False,
                )
                nc.tensor.matmul(
                    t3g[:, j * 512 : j * 512 + NCOL],
                    XXT1[:, n * F : (n + 1) * F],
                    MOV1,
                    start=False,
                    stop=True,
                )
            ts = work.tile([128, NG * NCOL], db, tag="ts")
            nc.scalar.copy(
                ts[:].rearrange("p (n c) -> p n c", n=NG),
                t3g[:].rearrange("p (n c) -> p n c", c=512)[:, :, 0:NCOL],
            )
            zg = work.tile([128, NG * NCOL], db, tag="zg")
            nc.vector.tensor_tensor(
                zg[:],
                ts[:],
                WALL[:, g * NG * NCOL : (g + 1) * NG * NCOL],
                op=mult,
            )
            for j in range(NG):
                n = g * NG + j
                sc = work.tile([128, NCOL], db, tag="sc")
                nc.vector.tensor_scalar(
                    out=sc[:],
                    in0=zg[:, j * NCOL : (j + 1) * NCOL],
                    scalar1=1.0,
                    scalar2=0.0,
                    op0=mult,
                    op1=add,
                    accum_out=ACCF[:, n : n + 1],
                )
            if g % 2 == 1:
                h = g // 2
                sl = slice(h * 8, (h + 1) * 8)
                nc.vector.tensor_tensor(OUT[:, sl], ACCF[:, sl], ACCL[:, sl], op=add)
                nc.sync.dma_start(out_d.ap()[:, sl], OUT[:, sl])

    nc.compile()
    return nc


def _host_prep(x, sorted_indices, weights_max, w2, w1, U3, U2, U1):
    """Build per-core input maps (layout/gather/replication work only)."""
    import ml_dtypes

    bf16 = ml_dtypes.bfloat16
    x = np.ascontiguousarray(x, dtype=np.float32)
    si = np.asarray(sorted_indices).astype(np.int64)
    W3 = np.asarray(weights_max, dtype=np.float32)
    W2 = np.asarray(w2, dtype=np.float32)
    W1 = np.asarray(w1, dtype=np.float32)
    U3 = np.asarray(U3, dtype=np.float32)
    U2 = np.asarray(U2, dtype=np.float32)
    U1 = np.asarray(U1, dtype=np.float32)

    U3r = U3.reshape(P3, L, L, L)  # [p, m1, m2, k]
    U3m = U3r.transpose(2, 3, 0, 1).reshape(L * L, P3 * L)  # [(m2,k), (p,m1)]
    U2m = U2.reshape(P2, L * L).T  # [(m2,k), q]
    movf = np.concatenate([U2m, U3m], axis=1)  # [(m2,k), 4+368]
    mov = np.ascontiguousarray(
        movf.reshape(2, 128, NCOL).transpose(1, 0, 2).reshape(128, 2 * NCOL).astype(bf16)
    )

    pidx = np.arange(128)
    in_maps = []
    for c in range(NCORES):
        sl = slice(c * BS, (c + 1) * BS)
        sic = si[sl]
        xc = x[sl]  # [16, 128, 16]
        xsb = xc.transpose(1, 0, 2).reshape(128, BS * L)
        cw3 = W3[sic].transpose(2, 0, 1).reshape(128, BS * P3)
        w2s = W2[sic].transpose(2, 0, 1).reshape(128, BS * P2)
        w1T = W1[sic][:, 0, :].T  # [128f, 16n]
        u1wm = (w1T[:, :, None] * U1.reshape(1, 1, L)).reshape(128, BS * L)
        ws = np.ascontiguousarray(
            np.concatenate([xsb, cw3, w2s, u1wm], axis=1).astype(bf16)
        )
        # replicated (m2,k)-partition layouts: xnf[l, (n,f)] = x[n,f,l]
        xnf = np.ascontiguousarray(
            xc.transpose(2, 0, 1).reshape(L, BS * F).astype(bf16)
        )
        xk = np.ascontiguousarray(xnf[pidx & 15])
        xm0 = np.ascontiguousarray(xnf[pidx >> 4])
        xm1 = np.ascontiguousarray(xnf[8 + (pidx >> 4)])
        in_maps.append(
            {"wsmall": ws, "mov": mov, "xk": xk, "xm0": xm0, "xm1": xm1}
        )
    return in_maps


def _get_nc():
    key = ("nc", os.environ.get("KERNEL_W_POOL", "2,3"))
    if key not in _CACHE:
        _CACHE[key] = _build_program(key)
    return _CACHE[key]


def kernel(
    x,
    bincount,
    sorted_indices,
    weights_max,
    w2,
    w1,
    U3,
    U2,
    U1,
    _trace=False,
):
    from concourse.bass_utils import run_bass_kernel_spmd

    nc = _get_nc()
    in_maps = _host_prep(x, sorted_indices, weights_max, w2, w1, U3, U2, U1)
    res = run_bass_kernel_spmd(
        nc, in_maps, core_ids=list(range(NCORES)), trace=_trace
    )
    outs = [res.results[c]["outT"] for c in range(NCORES)]  # each [128f, 16n]
    full = np.concatenate([o.T for o in outs], axis=0)  # [128, 128]
    out = np.ascontiguousarray(full, dtype=np.float32)
    if _trace:
        return out, res
    return out


# revision 11
# speedup vs baseline: 1.0612x; 1.0612x over previous
"""Trainium2 Bass kernel for nn_Contraction_627065225897 (MACE-style symmetric
contraction with per-element (MoE-routed) weights).

Math (per atom n with element e = sorted_indices[n], channel f):
  out[n,f] = sum_p W3[e,p,f] * T3[n,f,p]  +  sum_q W2[e,q,f] * T2[n,f,q]
           + W1[e,0,f] * T1[n,f]
  T3[n,f,p]   = sum_{m1,m2,k} U3[p,m1,m2,k] x_m1 x_m2 x_k   (+ lower orders)

Device strategy (per core, 16 atoms, f=128 on output partitions):
  - host ships x pre-replicated in (m2,k)-partition layout (XK/XM, bf16) in
    two half-chunks each, so xxT[(m2,k), (n,f)] = XM * XK is a packed
    elementwise bf16 multiply on DVE (2x) -- no outer products, no PE
    transposes, no PSUM evac for xx
  - PE matmuls (bf16): t3[f, 4q+368pm] = xxT_n.T @ MOV, 2 chained per atom
  - W[f, n, 372] = [w2s | W3[e_n] (x) x] built with stride-0 broadcast APs
    (DVE/GPSIMD split), bf16
  - Act evacuates t3 PSUM->SBUF per 4-atom group, converting to bf16
  - fused reduce per group: Z = ts * W (packed bf16, DVE 2x) then per-atom
    tensor_scalar accumulate (DVE 4x) -> ACCF
  - linear term: M = x * (U1 (x) W1) on GPSIMD + grouped tensor_reduce (DVE)
  - output written in two halves to overlap the final DMA with compute
Sharding: data-parallel over atoms, 16 atoms/core on 8 cores; per-element
weights gathered host-side by sorted_indices (routing) and replicated.
"""

import os
import sys
from contextlib import ExitStack

import numpy as np

if "/opt/trn_rl_repo" not in sys.path:
    sys.path.insert(0, "/opt/trn_rl_repo")

B, F, L = 128, 128, 16
E = 10
P3, P2, P1 = 23, 4, 1
NCORES = 8
BS = B // NCORES  # atoms per core = 16
NPM = P3 * L  # 368 cubic columns (p-major, m1-minor)
NCOL = P2 + NPM  # 372 moving columns: [quad(4) | cubic(368)]
NCOL2 = NCOL + L  # 388: fused-reduce row also carries the linear partials
NG = 4  # atoms per t3 PSUM group (4 banks/group)
NF = BS * F  # 2048 (n,f) columns in replicated layout
HF = NF // 2  # half of the (n,f) columns (atoms 0-7 / 8-15)
# wsmall packing: [ xsb(256) | cw3(368) | w2s(64) | u1wm(256) ]
WS_XSB, WS_CW3, WS_W2S, WS_U1 = 0, 256, 624, 688
WS_TOT = 944

_CACHE = {}


def _build_program(cfg_key):
    import concourse.bass as bass
    import concourse.mybir as mybir
    import concourse.tile as tile
    from concourse import bacc

    dt = mybir.dt.float32
    db = mybir.dt.bfloat16
    nc = bacc.Bacc("TRN2", target_bir_lowering=False, debug=False)

    # W-build batches (of 4 atoms) on GPSIMD instead of DVE
    w_pool = set(
        int(s) for s in os.environ.get("KERNEL_W_POOL", "2,3").split(",") if s
    )

    ws_d = nc.dram_tensor("wsmall", [128, WS_TOT + 2 * NCOL], db, kind="ExternalInput")
    xk_d = nc.dram_tensor("xk", [128, NF], db, kind="ExternalInput")
    xm0_d = nc.dram_tensor("xm0", [128, NF], db, kind="ExternalInput")
    xm1_d = nc.dram_tensor("xm1", [128, NF], db, kind="ExternalInput")
    out_d = nc.dram_tensor("outT", [128, BS], dt, kind="ExternalOutput")

    mult = mybir.AluOpType.mult
    add = mybir.AluOpType.add

    with tile.TileContext(nc) as tc, ExitStack() as ctx:
        const = ctx.enter_context(tc.tile_pool(name="const", bufs=1))
        work = ctx.enter_context(tc.tile_pool(name="work", bufs=3))
        ps_t3 = ctx.enter_context(
            tc.tile_pool(name="ps_t3", bufs=2, space=bass.MemorySpace.PSUM)
        )

        WS = const.tile([128, WS_TOT + 2 * NCOL], db)
        XK = const.tile([128, NF], db)
        XM0 = const.tile([128, NF], db)
        XM1 = const.tile([128, NF], db)
        MOV = WS[:, WS_TOT : WS_TOT + 2 * NCOL]

        # All DMAs on the SP queue, ordered by need time; the replicated-x
        # tensors are split in half-chunks so the first matmuls start early.
        nc.sync.dma_start(XK[:, 0:HF], xk_d.ap()[:, 0:HF])
        nc.sync.dma_start(XM0[:, 0:HF], xm0_d.ap()[:, 0:HF])
        nc.sync.dma_start(XM1[:, 0:HF], xm1_d.ap()[:, 0:HF])
        nc.sync.dma_start(WS[:], ws_d.ap())
        nc.sync.dma_start(XK[:, HF:NF], xk_d.ap()[:, HF:NF])
        nc.sync.dma_start(XM0[:, HF:NF], xm0_d.ap()[:, HF:NF])
        nc.sync.dma_start(XM1[:, HF:NF], xm1_d.ap()[:, HF:NF])

        # PE p-state warm-up: dummy matmuls on a memset tile keep the PE busy
        # from t~0.7us so the real matmuls run at full clock (3us ramp).
        WARM = const.tile([128, 128], db)
        nc.gpsimd.memset(WARM[:], 0.0)
        warm_ps = ps_t3.tile([128, NG * 512], dt, tag="t3g")
        wmov = WARM[:].unsqueeze(1).broadcast_to((128, 4, 128))
        for _ in range(13):
            nc.tensor.matmul(warm_ps[:, 0:512], WARM[:], wmov, start=True, stop=True)

        XSB = WS[:, WS_XSB : WS_XSB + BS * L]
        CW3 = WS[:, WS_CW3 : WS_CW3 + BS * P3]
        U1WM = WS[:, WS_U1 : WS_U1 + BS * L]
        MOV0 = MOV[:, 0:NCOL]
        MOV1 = MOV[:, NCOL : 2 * NCOL]

        WALL = const.tile([128, BS * NCOL], db)
        WALL_v = WALL[:].rearrange("p (n c) -> p n c", n=BS)
        # quad weight cols [0:4) copied from wsmall (tiny DVE copy)
        wq = work.tile([128, BS * P2], db, tag="wq")
        nc.vector.tensor_scalar(
            out=WALL_v[:, :, 0:P2],
            in0=WS[:, WS_W2S : WS_W2S + BS * P2].rearrange("p (n c) -> p n c", n=BS),
            scalar1=1.0,
            scalar2=0.0,
            op0=mult,
            op1=add,
        )

        XXT0 = const.tile([128, NF], db)  # xxT half 0: partitions (m2<8, k)
        XXT1 = const.tile([128, NF], db)  # xxT half 1: partitions (m2>=8, k)
        ACCF = const.tile([128, BS], dt)  # fused quad+cubic+linear accums

        # ---- W[f, n, 4+(p,m1)] = W3[e_n,p,f] * x[n,f,m1] (stride-0 APs) ----
        def build_w(b):
            wv = WALL_v[:, b * NG : (b + 1) * NG, P2:NCOL].rearrange(
                "p n (a b) -> p n a b", a=P3
            )
            c0 = (
                CW3[:, b * NG * P3 : (b + 1) * NG * P3]
                .rearrange("p (n a) -> p n a", n=NG)
                .unsqueeze(3)
                .broadcast_to((128, NG, P3, L))
            )
            x0 = (
                XSB[:, b * NG * L : (b + 1) * NG * L]
                .rearrange("p (n a) -> p n a", n=NG)
                .unsqueeze(2)
                .broadcast_to((128, NG, P3, L))
            )
            weng = nc.gpsimd if b in w_pool else nc.vector
            weng.tensor_tensor(wv, c0, x0, op=mult)

        # xxT for one half h of the atoms (8 atoms = 1024 cols)
        def build_xxt(h):
            sl = slice(h * HF, (h + 1) * HF)
            nc.vector.tensor_tensor(XXT0[:, sl], XM0[:, sl], XK[:, sl], op=mult)
            nc.vector.tensor_tensor(XXT1[:, sl], XM1[:, sl], XK[:, sl], op=mult)

        # Pool builds its W batches early; DVE interleaves xxT and W
        for b in sorted(w_pool):
            build_w(b)
        build_xxt(0)
        for b in range(BS // NG):
            if b not in w_pool:
                build_w(b)
        build_xxt(1)

        # ---- PE matmuls -> Act group evac (bf16) -> DVE Z + 4x accumulate ----
        for g in range(BS // NG):
            t3g = ps_t3.tile([128, NG * 512], dt, tag="t3g")
            for j in range(NG):
                n = g * NG + j
                nc.tensor.matmul(
                    t3g[:, j * 512 : j * 512 + NCOL],
                    XXT0[:, n * F : (n + 1) * F],
                    MOV0,
                    start=True,
                    stop=False,
                )
                nc.tensor.matmul(
                    t3g[:, j * 512 : j * 512 + NCOL],
                    XXT1[:, n * F : (n + 1) * F],
                    MOV1,
                    start=False,
                    stop=True,
                )
            ts = work.tile([128, NG * NCOL], db, tag="ts")
            nc.scalar.copy(
                ts[:].rearrange("p (n c) -> p n c", n=NG),
                t3g[:].rearrange("p (n c) -> p n c", c=512)[:, :, 0:NCOL],
            )
            zg = work.tile([128, NG * NCOL2], db, tag="zg")
            zg_v = zg[:].rearrange("p (n c) -> p n c", n=NG)
            nc.vector.tensor_tensor(
                zg_v[:, :, 0:NCOL],
                ts[:].rearrange("p (n c) -> p n c", n=NG),
                WALL_v[:, g * NG : (g + 1) * NG, :],
                op=mult,
            )
            # linear partials x*(U1 (x) W1) for this group's atoms (GPSIMD)
            nc.gpsimd.tensor_tensor(
                zg_v[:, :, NCOL:NCOL2],
                XSB[:, g * NG * L : (g + 1) * NG * L].rearrange(
                    "p (n l) -> p n l", n=NG
                ),
                U1WM[:, g * NG * L : (g + 1) * NG * L].rearrange(
                    "p (n l) -> p n l", n=NG
                ),
                op=mult,
            )
            for j in range(NG):
                n = g * NG + j
                sc = work.tile([128, NCOL2], db, tag="sc")
                nc.vector.tensor_scalar(
                    out=sc[:],
                    in0=zg[:, j * NCOL2 : (j + 1) * NCOL2],
                    scalar1=1.0,
                    scalar2=0.0,
                    op0=mult,
                    op1=add,
                    accum_out=ACCF[:, n : n + 1],
                )
            if g % 2 == 1:
                h = g // 2
                sl = slice(h * 8, (h + 1) * 8)
                nc.sync.dma_start(out_d.ap()[:, sl], ACCF[:, sl])

    nc.compile()
    return nc


def _host_prep(x, sorted_indices, weights_max, w2, w1, U3, U2, U1):
    """Build per-core input maps (layout/gather/replication work only)."""
    import ml_dtypes

    bf16 = ml_dtypes.bfloat16
    x = np.ascontiguousarray(x, dtype=np.float32)
    si = np.asarray(sorted_indices).astype(np.int64)
    W3 = np.asarray(weights_max, dtype=np.float32)
    W2 = np.asarray(w2, dtype=np.float32)
    W1 = np.asarray(w1, dtype=np.float32)
    U3 = np.asarray(U3, dtype=np.float32)
    U2 = np.asarray(U2, dtype=np.float32)
    U1 = np.asarray(U1, dtype=np.float32)

    U3r = U3.reshape(P3, L, L, L)  # [p, m1, m2, k]
    U3m = U3r.transpose(2, 3, 0, 1).reshape(L * L, P3 * L)  # [(m2,k), (p,m1)]
    U2m = U2.reshape(P2, L * L).T  # [(m2,k), q]
    movf = np.concatenate([U2m, U3m], axis=1)  # [(m2,k), 4+368]
    mov = np.ascontiguousarray(
        movf.reshape(2, 128, NCOL).transpose(1, 0, 2).reshape(128, 2 * NCOL).astype(bf16)
    )

    pidx = np.arange(128)
    in_maps = []
    for c in range(NCORES):
        sl = slice(c * BS, (c + 1) * BS)
        sic = si[sl]
        xc = x[sl]  # [16, 128, 16]
        xsb = xc.transpose(1, 0, 2).reshape(128, BS * L)
        cw3 = W3[sic].transpose(2, 0, 1).reshape(128, BS * P3)
        w2s = W2[sic].transpose(2, 0, 1).reshape(128, BS * P2)
        w1T = W1[sic][:, 0, :].T  # [128f, 16n]
        u1wm = (w1T[:, :, None] * U1.reshape(1, 1, L)).reshape(128, BS * L)
        ws = np.ascontiguousarray(
            np.concatenate(
                [xsb, cw3, w2s, u1wm, mov.astype(np.float32)], axis=1
            ).astype(bf16)
        )
        # replicated (m2,k)-partition layouts: xnf[l, (n,f)] = x[n,f,l]
        xnf = np.ascontiguousarray(
            xc.transpose(2, 0, 1).reshape(L, BS * F).astype(bf16)
        )
        xk = np.ascontiguousarray(xnf[pidx & 15])
        xm0 = np.ascontiguousarray(xnf[pidx >> 4])
        xm1 = np.ascontiguousarray(xnf[8 + (pidx >> 4)])
        in_maps.append({"wsmall": ws, "xk": xk, "xm0": xm0, "xm1": xm1})
    return in_maps


def _get_nc():
    key = ("nc", os.environ.get("KERNEL_W_POOL", "2,3"))
    if key not in _CACHE:
        _CACHE[key] = _build_program(key)
    return _CACHE[key]


def kernel(
    x,
    bincount,
    sorted_indices,
    weights_max,
    w2,
    w1,
    U3,
    U2,
    U1,
    _trace=False,
):
    from concourse.bass_utils import run_bass_kernel_spmd

    nc = _get_nc()
    in_maps = _host_prep(x, sorted_indices, weights_max, w2, w1, U3, U2, U1)
    res = run_bass_kernel_spmd(
        nc, in_maps, core_ids=list(range(NCORES)), trace=_trace
    )
    outs = [res.results[c]["outT"] for c in range(NCORES)]  # each [128f, 16n]
    full = np.concatenate([o.T for o in outs], axis=0)  # [128, 128]
    out = np.ascontiguousarray(full, dtype=np.float32)
    if _trace:
        return out, res
    return out


# revision 13
# speedup vs baseline: 1.1154x; 1.0510x over previous
"""Trainium2 Bass kernel for nn_Contraction_627065225897 (MACE-style symmetric
contraction with per-element (MoE-routed) weights).

Math (per atom n with element e = sorted_indices[n], channel f):
  out[n,f] = sum_p W3[e,p,f] * T3[n,f,p]  +  sum_q W2[e,q,f] * T2[n,f,q]
           + W1[e,0,f] * T1[n,f]
  T3[n,f,p]   = sum_{m1,m2,k} U3[p,m1,m2,k] x_m1 x_m2 x_k   (+ lower orders)

Device strategy (per core, 16 atoms, f=128 on output partitions):
  - host ships x pre-replicated in (m2,k)-partition layout (XK/XM, bf16) in
    two half-chunks each, so xxT[(m2,k), (n,f)] = XM * XK is a packed
    elementwise bf16 multiply on DVE (2x) -- no outer products, no PE
    transposes, no PSUM evac for xx
  - PE matmuls (bf16): t3[f, 4q+368pm] = xxT_n.T @ MOV, 2 chained per atom
  - W[f, n, 372] = [w2s | W3[e_n] (x) x] built with stride-0 broadcast APs
    (DVE/GPSIMD split), bf16
  - Act evacuates t3 PSUM->SBUF per 4-atom group, converting to bf16
  - fused reduce per group: Z = ts * W (packed bf16, DVE 2x) then per-atom
    tensor_scalar accumulate (DVE 4x) -> ACCF
  - linear term: M = x * (U1 (x) W1) on GPSIMD + grouped tensor_reduce (DVE)
  - output written in two halves to overlap the final DMA with compute
Sharding: data-parallel over atoms, 16 atoms/core on 8 cores; per-element
weights gathered host-side by sorted_indices (routing) and replicated.
"""

import os
import sys
from contextlib import ExitStack

import numpy as np

if "/opt/trn_rl_repo" not in sys.path:
    sys.path.insert(0, "/opt/trn_rl_repo")

B, F, L = 128, 128, 16
E = 10
P3, P2, P1 = 23, 4, 1
NCORES = 8
BS = B // NCORES  # atoms per core = 16
NPM = P3 * L  # 368 cubic columns (p-major, m1-minor)
NCOL = P2 + NPM  # 372 moving columns: [quad(4) | cubic(368)]
NCOL2 = NCOL + L  # 388: fused-reduce row also carries the linear partials
NG = 4  # atoms per t3 PSUM group (4 banks/group)
NF = BS * F  # 2048 (n,f) columns in replicated layout
HF = NF // 2  # half of the (n,f) columns (atoms 0-7 / 8-15)
# wsmall packing: [ xsb(256) | cw3(368) | w2s(64) | u1wm(256) ]
WS_XSB, WS_CW3, WS_W2S, WS_U1 = 0, 256, 624, 688
WS_TOT = 944

_CACHE = {}


def _build_program(cfg_key):
    import concourse.bass as bass
    import concourse.mybir as mybir
    import concourse.tile as tile
    from concourse import bacc

    dt = mybir.dt.float32
    db = mybir.dt.bfloat16
    nc = bacc.Bacc("TRN2", target_bir_lowering=False, debug=False)

    # W-build batches (of 4 atoms) on GPSIMD instead of DVE
    w_pool = set(
        int(s) for s in os.environ.get("KERNEL_W_POOL", "2,3").split(",") if s
    )

    ws_d = nc.dram_tensor("wsmall", [128, WS_TOT + 2 * NCOL], db, kind="ExternalInput")
    xk_d = nc.dram_tensor("xk", [128, NF], db, kind="ExternalInput")
    xm0_d = nc.dram_tensor("xm0", [128, NF], db, kind="ExternalInput")
    xm1_d = nc.dram_tensor("xm1", [128, NF], db, kind="ExternalInput")
    out_d = nc.dram_tensor("outT", [128, BS], dt, kind="ExternalOutput")

    mult = mybir.AluOpType.mult
    add = mybir.AluOpType.add

    with tile.TileContext(nc) as tc, ExitStack() as ctx:
        const = ctx.enter_context(tc.tile_pool(name="const", bufs=1))
        work = ctx.enter_context(tc.tile_pool(name="work", bufs=3))
        ps_t3 = ctx.enter_context(
            tc.tile_pool(name="ps_t3", bufs=2, space=bass.MemorySpace.PSUM)
        )

        WS = const.tile([128, WS_TOT + 2 * NCOL], db)
        XK = const.tile([128, NF], db)
        XM0 = const.tile([128, NF], db)
        XM1 = const.tile([128, NF], db)
        MOV = WS[:, WS_TOT : WS_TOT + 2 * NCOL]

        # All DMAs on the SP queue, ordered by need time; the replicated-x
        # tensors are split in half-chunks so the first matmuls start early.
        nc.sync.dma_start(XK[:, 0:HF], xk_d.ap()[:, 0:HF])
        nc.sync.dma_start(XM0[:, 0:HF], xm0_d.ap()[:, 0:HF])
        nc.sync.dma_start(XM1[:, 0:HF], xm1_d.ap()[:, 0:HF])
        nc.sync.dma_start(WS[:], ws_d.ap())
        nc.sync.dma_start(XK[:, HF:NF], xk_d.ap()[:, HF:NF])
        nc.sync.dma_start(XM0[:, HF:NF], xm0_d.ap()[:, HF:NF])
        nc.sync.dma_start(XM1[:, HF:NF], xm1_d.ap()[:, HF:NF])

        # PE p-state warm-up: dummy matmuls on a memset tile keep the PE busy
        # from t~0.7us so the real matmuls run at full clock (3us ramp).
        WARM = const.tile([128, 128], db)
        nc.gpsimd.memset(WARM[:], 0.0)
        warm_ps = ps_t3.tile([128, NG * 512], dt, tag="t3g")
        wmov = WARM[:].unsqueeze(1).broadcast_to((128, 4, 128))
        for _ in range(13):
            nc.tensor.matmul(warm_ps[:, 0:512], WARM[:], wmov, start=True, stop=True)

        XSB = WS[:, WS_XSB : WS_XSB + BS * L]
        CW3 = WS[:, WS_CW3 : WS_CW3 + BS * P3]
        U1WM = WS[:, WS_U1 : WS_U1 + BS * L]
        MOV0 = MOV[:, 0:NCOL]
        MOV1 = MOV[:, NCOL : 2 * NCOL]

        WALL = const.tile([128, BS * NCOL], db)
        WALL_v = WALL[:].rearrange("p (n c) -> p n c", n=BS)
        # quad weight cols [0:4) copied from wsmall (tiny DVE copy)
        wq = work.tile([128, BS * P2], db, tag="wq")
        nc.vector.tensor_scalar(
            out=WALL_v[:, :, 0:P2],
            in0=WS[:, WS_W2S : WS_W2S + BS * P2].rearrange("p (n c) -> p n c", n=BS),
            scalar1=1.0,
            scalar2=0.0,
            op0=mult,
            op1=add,
        )

        XXT0 = const.tile([128, NF], db)  # xxT half 0: partitions (m2<8, k)
        XXT1 = const.tile([128, NF], db)  # xxT half 1: partitions (m2>=8, k)
        ACCF = const.tile([128, BS], dt)  # fused quad+cubic+linear accums
        ZG = [
            const.tile([128, NG * NCOL2], db, name=f"zg{g}")
            for g in range(BS // NG)
        ]

        # linear partials x*(U1 (x) W1) per group, on GPSIMD BEFORE its W
        # builds so the fused accumulates are not queue-blocked behind them
        for g in range(BS // NG):
            nc.gpsimd.tensor_tensor(
                ZG[g][:].rearrange("p (n c) -> p n c", n=NG)[:, :, NCOL:NCOL2],
                XSB[:, g * NG * L : (g + 1) * NG * L].rearrange(
                    "p (n l) -> p n l", n=NG
                ),
                U1WM[:, g * NG * L : (g + 1) * NG * L].rearrange(
                    "p (n l) -> p n l", n=NG
                ),
                op=mult,
            )

        # ---- W[f, n, 4+(p,m1)] = W3[e_n,p,f] * x[n,f,m1] (stride-0 APs) ----
        def build_w(b):
            wv = WALL_v[:, b * NG : (b + 1) * NG, P2:NCOL].rearrange(
                "p n (a b) -> p n a b", a=P3
            )
            c0 = (
                CW3[:, b * NG * P3 : (b + 1) * NG * P3]
                .rearrange("p (n a) -> p n a", n=NG)
                .unsqueeze(3)
                .broadcast_to((128, NG, P3, L))
            )
            x0 = (
                XSB[:, b * NG * L : (b + 1) * NG * L]
                .rearrange("p (n a) -> p n a", n=NG)
                .unsqueeze(2)
                .broadcast_to((128, NG, P3, L))
            )
            weng = nc.gpsimd if b in w_pool else nc.vector
            weng.tensor_tensor(wv, c0, x0, op=mult)

        # xxT for one half h of the atoms (8 atoms = 1024 cols)
        def build_xxt(h):
            sl = slice(h * HF, (h + 1) * HF)
            nc.vector.tensor_tensor(XXT0[:, sl], XM0[:, sl], XK[:, sl], op=mult)
            nc.vector.tensor_tensor(XXT1[:, sl], XM1[:, sl], XK[:, sl], op=mult)

        # Pool builds its W batches early; DVE interleaves xxT and W
        for b in sorted(w_pool):
            build_w(b)
        build_xxt(0)
        for b in range(BS // NG):
            if b not in w_pool:
                build_w(b)
        build_xxt(1)

        # ---- PE matmuls -> Act group evac (bf16) -> DVE Z + 4x accumulate ----
        for g in range(BS // NG):
            t3g = ps_t3.tile([128, NG * 512], dt, tag="t3g")
            for j in range(NG):
                n = g * NG + j
                nc.tensor.matmul(
                    t3g[:, j * 512 : j * 512 + NCOL],
                    XXT0[:, n * F : (n + 1) * F],
                    MOV0,
                    start=True,
                    stop=False,
                )
                nc.tensor.matmul(
                    t3g[:, j * 512 : j * 512 + NCOL],
                    XXT1[:, n * F : (n + 1) * F],
                    MOV1,
                    start=False,
                    stop=True,
                )
            ts = work.tile([128, NG * NCOL], db, tag="ts")
            nc.scalar.copy(
                ts[:].rearrange("p (n c) -> p n c", n=NG),
                t3g[:].rearrange("p (n c) -> p n c", c=512)[:, :, 0:NCOL],
            )
            zg = ZG[g]
            zg_v = zg[:].rearrange("p (n c) -> p n c", n=NG)
            nc.vector.tensor_tensor(
                zg_v[:, :, 0:NCOL],
                ts[:].rearrange("p (n c) -> p n c", n=NG),
                WALL_v[:, g * NG : (g + 1) * NG, :],
                op=mult,
            )
            for j in range(NG):
                n = g * NG + j
                sc = work.tile([128, NCOL2], db, tag="sc")
                nc.vector.tensor_scalar(
                    out=sc[:],
                    in0=zg[:, j * NCOL2 : (j + 1) * NCOL2],
                    scalar1=1.0,
                    scalar2=0.0,
                    op0=mult,
                    op1=add,
                    accum_out=ACCF[:, n : n + 1],
                )
            if g % 2 == 1:
                h = g // 2
                sl = slice(h * 8, (h + 1) * 8)
                nc.sync.dma_start(out_d.ap()[:, sl], ACCF[:, sl])

    nc.compile()
    return nc


def _host_prep(x, sorted_indices, weights_max, w2, w1, U3, U2, U1):
    """Build per-core input maps (layout/gather/replication work only)."""
    import ml_dtypes

    bf16 = ml_dtypes.bfloat16
    x = np.ascontiguousarray(x, dtype=np.float32)
    si = np.asarray(sorted_indices).astype(np.int64)
    W3 = np.asarray(weights_max, dtype=np.float32)
    W2 = np.asarray(w2, dtype=np.float32)
    W1 = np.asarray(w1, dtype=np.float32)
    U3 = np.asarray(U3, dtype=np.float32)
    U2 = np.asarray(U2, dtype=np.float32)
    U1 = np.asarray(U1, dtype=np.float32)

    U3r = U3.reshape(P3, L, L, L)  # [p, m1, m2, k]
    U3m = U3r.transpose(2, 3, 0, 1).reshape(L * L, P3 * L)  # [(m2,k), (p,m1)]
    U2m = U2.reshape(P2, L * L).T  # [(m2,k), q]
    movf = np.concatenate([U2m, U3m], axis=1)  # [(m2,k), 4+368]
    mov = np.ascontiguousarray(
        movf.reshape(2, 128, NCOL).transpose(1, 0, 2).reshape(128, 2 * NCOL).astype(bf16)
    )

    pidx = np.arange(128)
    in_maps = []
    for c in range(NCORES):
        sl = slice(c * BS, (c + 1) * BS)
        sic = si[sl]
        xc = x[sl]  # [16, 128, 16]
        xsb = xc.transpose(1, 0, 2).reshape(128, BS * L)
        cw3 = W3[sic].transpose(2, 0, 1).reshape(128, BS * P3)
        w2s = W2[sic].transpose(2, 0, 1).reshape(128, BS * P2)
        w1T = W1[sic][:, 0, :].T  # [128f, 16n]
        u1wm = (w1T[:, :, None] * U1.reshape(1, 1, L)).reshape(128, BS * L)
        ws = np.ascontiguousarray(
            np.concatenate(
                [xsb, cw3, w2s, u1wm, mov.astype(np.float32)], axis=1
            ).astype(bf16)
        )
        # replicated (m2,k)-partition layouts: xnf[l, (n,f)] = x[n,f,l]
        xnf = np.ascontiguousarray(
            xc.transpose(2, 0, 1).reshape(L, BS * F).astype(bf16)
        )
        xk = np.ascontiguousarray(xnf[pidx & 15])
        xm0 = np.ascontiguousarray(xnf[pidx >> 4])
        xm1 = np.ascontiguousarray(xnf[8 + (pidx >> 4)])
        in_maps.append({"wsmall": ws, "xk": xk, "xm0": xm0, "xm1": xm1})
    return in_maps


def _get_nc():
    key = ("nc", os.environ.get("KERNEL_W_POOL", "2,3"))
    if key not in _CACHE:
        _CACHE[key] = _build_program(key)
    return _CACHE[key]


def kernel(
    x,
    bincount,
    sorted_indices,
    weights_max,
    w2,
    w1,
    U3,
    U2,
    U1,
    _trace=False,
):
    from concourse.bass_utils import run_bass_kernel_spmd

    nc = _get_nc()
    in_maps = _host_prep(x, sorted_indices, weights_max, w2, w1, U3, U2, U1)
    res = run_bass_kernel_spmd(
        nc, in_maps, core_ids=list(range(NCORES)), trace=_trace
    )
    outs = [res.results[c]["outT"] for c in range(NCORES)]  # each [128f, 16n]
    full = np.concatenate([o.T for o in outs], axis=0)  # [128, 128]
    out = np.ascontiguousarray(full, dtype=np.float32)
    if _trace:
        return out, res
    return out


# revision 14
# speedup vs baseline: 1.1162x; 1.0007x over previous
"""Trainium2 Bass kernel for nn_Contraction_627065225897 (MACE-style symmetric
contraction with per-element (MoE-routed) weights).

Math (per atom n with element e = sorted_indices[n], channel f):
  out[n,f] = sum_p W3[e,p,f] * T3[n,f,p]  +  sum_q W2[e,q,f] * T2[n,f,q]
           + W1[e,0,f] * T1[n,f]
  T3[n,f,p]   = sum_{m1,m2,k} U3[p,m1,m2,k] x_m1 x_m2 x_k   (+ lower orders)

Device strategy (per core, 16 atoms, f=128 on output partitions):
  - host ships x pre-replicated in (m2,k)-partition layout (XK/XM, bf16) in
    two half-chunks each, so xxT[(m2,k), (n,f)] = XM * XK is a packed
    elementwise bf16 multiply on DVE (2x) -- no outer products, no PE
    transposes, no PSUM evac for xx
  - PE matmuls (bf16): t3[f, 4q+368pm] = xxT_n.T @ MOV, 2 chained per atom
  - W[f, n, 372] = [w2s | W3[e_n] (x) x] built with stride-0 broadcast APs
    (DVE/GPSIMD split), bf16
  - Act evacuates t3 PSUM->SBUF per 4-atom group, converting to bf16
  - fused reduce per group: Z = ts * W (packed bf16, DVE 2x) then per-atom
    tensor_scalar accumulate (DVE 4x) -> ACCF
  - linear term: M = x * (U1 (x) W1) on GPSIMD + grouped tensor_reduce (DVE)
  - output written in two halves to overlap the final DMA with compute
Sharding: data-parallel over atoms, 16 atoms/core on 8 cores; per-element
weights gathered host-side by sorted_indices (routing) and replicated.
"""

import os
import sys
from contextlib import ExitStack

import numpy as np

if "/opt/trn_rl_repo" not in sys.path:
    sys.path.insert(0, "/opt/trn_rl_repo")

B, F, L = 128, 128, 16
E = 10
P3, P2, P1 = 23, 4, 1
NCORES = 8
BS = B // NCORES  # atoms per core = 16
NPM = P3 * L  # 368 cubic columns (p-major, m1-minor)
NCOL = P2 + NPM  # 372 moving columns: [quad(4) | cubic(368)]
NCOL2 = NCOL + L  # 388: fused-reduce row also carries the linear partials
NG = 4  # atoms per t3 PSUM group (4 banks/group)
NF = BS * F  # 2048 (n,f) columns in replicated layout
HF = NF // 2  # half of the (n,f) columns (atoms 0-7 / 8-15)
# wsmall packing: [ xsb(256) | cw3(368) | w2s(64) | u1wm(256) ]
WS_XSB, WS_CW3, WS_W2S, WS_U1 = 0, 256, 624, 688
WS_TOT = 944

_CACHE = {}


def _build_program(cfg_key):
    import concourse.bass as bass
    import concourse.mybir as mybir
    import concourse.tile as tile
    from concourse import bacc

    dt = mybir.dt.float32
    db = mybir.dt.bfloat16
    nc = bacc.Bacc("TRN2", target_bir_lowering=False, debug=False)

    # W-build batches (of 4 atoms) on GPSIMD instead of DVE
    w_pool = set(
        int(s) for s in os.environ.get("KERNEL_W_POOL", "2,3").split(",") if s
    )

    ws_d = nc.dram_tensor("wsmall", [128, WS_TOT + 2 * NCOL], db, kind="ExternalInput")
    xk_d = nc.dram_tensor("xk", [128, NF], db, kind="ExternalInput")
    xm0_d = nc.dram_tensor("xm0", [128, NF], db, kind="ExternalInput")
    xm1_d = nc.dram_tensor("xm1", [128, NF], db, kind="ExternalInput")
    out_d = nc.dram_tensor("outT", [128, BS], dt, kind="ExternalOutput")

    mult = mybir.AluOpType.mult
    add = mybir.AluOpType.add

    with tile.TileContext(nc) as tc, ExitStack() as ctx:
        const = ctx.enter_context(tc.tile_pool(name="const", bufs=1))
        work = ctx.enter_context(tc.tile_pool(name="work", bufs=3))
        ps_t3 = ctx.enter_context(
            tc.tile_pool(name="ps_t3", bufs=2, space=bass.MemorySpace.PSUM)
        )

        WS = const.tile([128, WS_TOT + 2 * NCOL], db)
        XK = const.tile([128, NF], db)
        XM0 = const.tile([128, NF], db)
        XM1 = const.tile([128, NF], db)
        MOV = WS[:, WS_TOT : WS_TOT + 2 * NCOL]

        # All DMAs on the SP queue, ordered by need time; the replicated-x
        # tensors are split in half-chunks so the first matmuls start early.
        nc.sync.dma_start(XK[:, 0:HF], xk_d.ap()[:, 0:HF])
        nc.sync.dma_start(XM0[:, 0:HF], xm0_d.ap()[:, 0:HF])
        nc.sync.dma_start(WS[:], ws_d.ap())
        nc.sync.dma_start(XM1[:, 0:HF], xm1_d.ap()[:, 0:HF])
        nc.sync.dma_start(XK[:, HF:NF], xk_d.ap()[:, HF:NF])
        nc.sync.dma_start(XM0[:, HF:NF], xm0_d.ap()[:, HF:NF])
        nc.sync.dma_start(XM1[:, HF:NF], xm1_d.ap()[:, HF:NF])

        # PE p-state warm-up: dummy matmuls on a memset tile keep the PE busy
        # from t~0.7us so the real matmuls run at full clock (3us ramp).
        WARM = const.tile([128, 128], db)
        nc.gpsimd.memset(WARM[:], 0.0)
        warm_ps = ps_t3.tile([128, NG * 512], dt, tag="t3g")
        wmov = WARM[:].unsqueeze(1).broadcast_to((128, 4, 128))
        for _ in range(13):
            nc.tensor.matmul(warm_ps[:, 0:512], WARM[:], wmov, start=True, stop=True)

        XSB = WS[:, WS_XSB : WS_XSB + BS * L]
        CW3 = WS[:, WS_CW3 : WS_CW3 + BS * P3]
        U1WM = WS[:, WS_U1 : WS_U1 + BS * L]
        MOV0 = MOV[:, 0:NCOL]
        MOV1 = MOV[:, NCOL : 2 * NCOL]

        WALL = const.tile([128, BS * NCOL], db)
        WALL_v = WALL[:].rearrange("p (n c) -> p n c", n=BS)
        # quad weight cols [0:4) copied from wsmall (tiny DVE copy)
        wq = work.tile([128, BS * P2], db, tag="wq")
        nc.vector.tensor_scalar(
            out=WALL_v[:, :, 0:P2],
            in0=WS[:, WS_W2S : WS_W2S + BS * P2].rearrange("p (n c) -> p n c", n=BS),
            scalar1=1.0,
            scalar2=0.0,
            op0=mult,
            op1=add,
        )

        XXT0 = const.tile([128, NF], db)  # xxT half 0: partitions (m2<8, k)
        XXT1 = const.tile([128, NF], db)  # xxT half 1: partitions (m2>=8, k)
        ACCF = const.tile([128, BS], dt)  # fused quad+cubic+linear accums
        ZG = [
            const.tile([128, NG * NCOL2], db, name=f"zg{g}")
            for g in range(BS // NG)
        ]

        # linear partials x*(U1 (x) W1) per group, on GPSIMD BEFORE its W
        # builds so the fused accumulates are not queue-blocked behind them
        for g in range(BS // NG):
            nc.gpsimd.tensor_tensor(
                ZG[g][:].rearrange("p (n c) -> p n c", n=NG)[:, :, NCOL:NCOL2],
                XSB[:, g * NG * L : (g + 1) * NG * L].rearrange(
                    "p (n l) -> p n l", n=NG
                ),
                U1WM[:, g * NG * L : (g + 1) * NG * L].rearrange(
                    "p (n l) -> p n l", n=NG
                ),
                op=mult,
            )

        # ---- W[f, n, 4+(p,m1)] = W3[e_n,p,f] * x[n,f,m1] (stride-0 APs) ----
        def build_w(b):
            wv = WALL_v[:, b * NG : (b + 1) * NG, P2:NCOL].rearrange(
                "p n (a b) -> p n a b", a=P3
            )
            c0 = (
                CW3[:, b * NG * P3 : (b + 1) * NG * P3]
                .rearrange("p (n a) -> p n a", n=NG)
                .unsqueeze(3)
                .broadcast_to((128, NG, P3, L))
            )
            x0 = (
                XSB[:, b * NG * L : (b + 1) * NG * L]
                .rearrange("p (n a) -> p n a", n=NG)
                .unsqueeze(2)
                .broadcast_to((128, NG, P3, L))
            )
            weng = nc.gpsimd if b in w_pool else nc.vector
            weng.tensor_tensor(wv, c0, x0, op=mult)

        # xxT half-ops, emitted in data-arrival order interleaved with the
        # DVE W builds so the in-order DVE queue never head-of-line blocks
        def build_xxt(h, which):
            sl = slice(h * HF, (h + 1) * HF)
            xm, xxt = (XM0, XXT0) if which == 0 else (XM1, XXT1)
            nc.vector.tensor_tensor(xxt[:, sl], xm[:, sl], XK[:, sl], op=mult)

        for b in sorted(w_pool):
            build_w(b)
        dve_w = [b for b in range(BS // NG) if b not in w_pool]
        build_xxt(0, 0)
        build_xxt(0, 1)
        if dve_w:
            build_w(dve_w[0])
        build_xxt(1, 0)
        build_xxt(1, 1)
        for b in dve_w[1:]:
            build_w(b)

        # ---- PE matmuls -> Act group evac (bf16) -> DVE Z + 4x accumulate ----
        for g in range(BS // NG):
            t3g = ps_t3.tile([128, NG * 512], dt, tag="t3g")
            for j in range(NG):
                n = g * NG + j
                nc.tensor.matmul(
                    t3g[:, j * 512 : j * 512 + NCOL],
                    XXT0[:, n * F : (n + 1) * F],
                    MOV0,
                    start=True,
                    stop=False,
                )
                nc.tensor.matmul(
                    t3g[:, j * 512 : j * 512 + NCOL],
                    XXT1[:, n * F : (n + 1) * F],
                    MOV1,
                    start=False,
                    stop=True,
                )
            ts = work.tile([128, NG * NCOL], db, tag="ts")
            nc.scalar.copy(
                ts[:].rearrange("p (n c) -> p n c", n=NG),
                t3g[:].rearrange("p (n c) -> p n c", c=512)[:, :, 0:NCOL],
            )
            zg = ZG[g]
            zg_v = zg[:].rearrange("p (n c) -> p n c", n=NG)
            nc.vector.tensor_tensor(
                zg_v[:, :, 0:NCOL],
                ts[:].rearrange("p (n c) -> p n c", n=NG),
                WALL_v[:, g * NG : (g + 1) * NG, :],
                op=mult,
            )
            for j in range(NG):
                n = g * NG + j
                sc = work.tile([128, NCOL2], db, tag="sc")
                nc.vector.tensor_scalar(
                    out=sc[:],
                    in0=zg[:, j * NCOL2 : (j + 1) * NCOL2],
                    scalar1=1.0,
                    scalar2=0.0,
                    op0=mult,
                    op1=add,
                    accum_out=ACCF[:, n : n + 1],
                )
            if g % 2 == 1:
                h = g // 2
                sl = slice(h * 8, (h + 1) * 8)
                nc.sync.dma_start(out_d.ap()[:, sl], ACCF[:, sl])

    nc.compile()
    return nc


def _host_prep(x, sorted_indices, weights_max, w2, w1, U3, U2, U1):
    """Build per-core input maps (layout/gather/replication work only)."""
    import ml_dtypes

    bf16 = ml_dtypes.bfloat16
    x = np.ascontiguousarray(x, dtype=np.float32)
    si = np.asarray(sorted_indices).astype(np.int64)
    W3 = np.asarray(weights_max, dtype=np.float32)
    W2 = np.asarray(w2, dtype=np.float32)
    W1 = np.asarray(w1, dtype=np.float32)
    U3 = np.asarray(U3, dtype=np.float32)
    U2 = np.asarray(U2, dtype=np.float32)
    U1 = np.asarray(U1, dtype=np.float32)

    U3r = U3.reshape(P3, L, L, L)  # [p, m1, m2, k]
    U3m = U3r.transpose(2, 3, 0, 1).reshape(L * L, P3 * L)  # [(m2,k), (p,m1)]
    U2m = U2.reshape(P2, L * L).T  # [(m2,k), q]
    movf = np.concatenate([U2m, U3m], axis=1)  # [(m2,k), 4+368]
    mov = np.ascontiguousarray(
        movf.reshape(2, 128, NCOL).transpose(1, 0, 2).reshape(128, 2 * NCOL).astype(bf16)
    )

    pidx = np.arange(128)
    in_maps = []
    for c in range(NCORES):
        sl = slice(c * BS, (c + 1) * BS)
        sic = si[sl]
        xc = x[sl]  # [16, 128, 16]
        xsb = xc.transpose(1, 0, 2).reshape(128, BS * L)
        cw3 = W3[sic].transpose(2, 0, 1).reshape(128, BS * P3)
        w2s = W2[sic].transpose(2, 0, 1).reshape(128, BS * P2)
        w1T = W1[sic][:, 0, :].T  # [128f, 16n]
        u1wm = (w1T[:, :, None] * U1.reshape(1, 1, L)).reshape(128, BS * L)
        ws = np.ascontiguousarray(
            np.concatenate(
                [xsb, cw3, w2s, u1wm, mov.astype(np.float32)], axis=1
            ).astype(bf16)
        )
        # replicated (m2,k)-partition layouts: xnf[l, (n,f)] = x[n,f,l]
        xnf = np.ascontiguousarray(
            xc.transpose(2, 0, 1).reshape(L, BS * F).astype(bf16)
        )
        xk = np.ascontiguousarray(xnf[pidx & 15])
        xm0 = np.ascontiguousarray(xnf[pidx >> 4])
        xm1 = np.ascontiguousarray(xnf[8 + (pidx >> 4)])
        in_maps.append({"wsmall": ws, "xk": xk, "xm0": xm0, "xm1": xm1})
    return in_maps


def _get_nc():
    key = ("nc", os.environ.get("KERNEL_W_POOL", "2,3"))
    if key not in _CACHE:
        _CACHE[key] = _build_program(key)
    return _CACHE[key]


def kernel(
    x,
    bincount,
    sorted_indices,
    weights_max,
    w2,
    w1,
    U3,
    U2,
    U1,
    _trace=False,
):
    from concourse.bass_utils import run_bass_kernel_spmd

    nc = _get_nc()
    in_maps = _host_prep(x, sorted_indices, weights_max, w2, w1, U3, U2, U1)
    res = run_bass_kernel_spmd(
        nc, in_maps, core_ids=list(range(NCORES)), trace=_trace
    )
    outs = [res.results[c]["outT"] for c in range(NCORES)]  # each [128f, 16n]
    full = np.concatenate([o.T for o in outs], axis=0)  # [128, 128]
    out = np.ascontiguousarray(full, dtype=np.float32)
    if _trace:
        return out, res
    return out
